# revision 1
# baseline (speedup 1.0000x reference)
"""Additive (Bahdanau) attention scoring kernel for Trainium2, 8-core SPMD.

Reference computation (B=16, S=4096, D=1024, all fp32):
    q      = target @ Wq.T                    # [B, D]
    k      = memory @ Wk.T                    # [B, S, D]
    scores = tanh(q[:, None, :] + k) @ v      # [B, S]
    out    = softmax(scores - 1e9 * mask, axis=-1)

Sharding: batch across the 8 cores (2 batches per core), weights replicated.

Host-side prep (layout only, no math): memory is transposed to [D, S] per
batch so the contraction dim lands on SBUF partitions, and its columns are
compacted to just the unmasked positions (padded with duplicates of the
first kept column to a 128-multiple, tail strip >= 256). Masked positions
contribute exactly 0 to the reference softmax (exp(-1e9) == 0 in fp32), so
skipping their k-matmul columns is algebraically exact.

Per-core device pipeline (python-unrolled, Tile-scheduled):
  - q^T via fp32r matmuls with target as the M=2 stationary and WqT as the
    N=512 moving operand (fp32r hard-faults the device for small moving N),
    transposed into per-partition bias layout through a DRAM bounce.
  - k^T tiles [e=128, s'=w] = WkT chunk.T @ memC chunk, fp32r accumulated
    over d. fp32r operands must be produced by a rounding compute op, so
    every DMA-landed operand gets a DVE cast into a separate f32r tile.
  - One ACT pass fuses the q-add and tanh (q as per-partition bias),
    writing f32r.
  - v-dot on the PE: psum[1, w] += v_chunk.T @ tanh_tile over the 8
    e-chunks; exp() applied in the ACT copy out of PSUM.
  - The exp strip is scattered back to full-S positions on device
    (DRAM bounce to [128, w/128], then indirect DMAs; duplicate pad
    indices are idempotent). scratch_full is zero-filled per batch, so
    masked positions are exactly 0.
  - Softmax finale per batch (no max-shift needed: |scores| <= sum|v| ~ 8,
    exp cannot overflow): [128, 32] esq load, mask multiply, free-dim
    reduce, ones-matmul partition reduce, reciprocal, per-partition scale.
"""

import os
from contextlib import ExitStack

import numpy as np

import concourse.tile as tile
from concourse import bacc, mybir
import concourse.bass as bass

B, S, D = 16, 4096, 1024
N_CORES = 8
NB = B // N_CORES  # batches per core
P = 128
DC = D // P        # contraction chunks
ET = D // P        # e tiles
SW = 512           # full strip width along compacted s
SQ = S // P        # 32: free dim of the [128, 32] softmax layout

F32 = mybir.dt.float32
F32R = mybir.dt.float32r
U32 = mybir.dt.uint32
AF = mybir.ActivationFunctionType

_CACHE = {}


def strip_widths(max_kept):
    """Strip widths covering max_kept compacted columns: full 512-wide strips
    plus a 128-granular tail of at least 256 (small moving-N fp32r matmuls
    hard-fault the device)."""
    total = max(512, ((max_kept + 127) // 128) * 128)
    widths = [SW] * (total // SW)
    rem = total % SW
    if rem:
        widths.append(max(256, rem))
    return tuple(widths)


def _build_program(stage, widths):
    """stage: 1 = dma+matmul+tanh only, 2 = +vdot/exp/scatter, 27 = full."""
    s_pad = sum(widths)
    nslot = s_pad // P  # indirect-scatter slots per batch

    nc = bacc.Bacc("TRN2", target_bir_lowering=False, debug=False)

    memC = nc.dram_tensor("memC", [NB, D, s_pad], F32, kind="ExternalInput").ap()
    wkT = nc.dram_tensor("wkT", [D, D], F32, kind="ExternalInput").ap()
    wqT = nc.dram_tensor("wqT", [D, D], F32, kind="ExternalInput").ap()
    tgtT = nc.dram_tensor("tgtT", [D, NB], F32, kind="ExternalInput").ap()
    vT = nc.dram_tensor("vT", [P, ET], F32, kind="ExternalInput").ap()
    keep = nc.dram_tensor("keep", [NB, P, SQ], F32, kind="ExternalInput").ap()
    idxs = nc.dram_tensor("idxs", [NB, nslot, P], U32, kind="ExternalInput").ap()
    out = nc.dram_tensor("out", [NB, P, SQ], F32, kind="ExternalOutput").ap()

    with tile.TileContext(nc) as tc, ExitStack() as ctx:
        consts = ctx.enter_context(tc.tile_pool(name="consts", bufs=1))
        mem_pool = ctx.enter_context(tc.tile_pool(name="mem", bufs=2))
        tt_pool = ctx.enter_context(tc.tile_pool(name="tt", bufs=4))
        strip_pool = ctx.enter_context(tc.tile_pool(name="strip", bufs=2))
        fin_pool = ctx.enter_context(tc.tile_pool(name="fin", bufs=2))
        kps_pool = ctx.enter_context(tc.tile_pool(name="kps", bufs=4, space="PSUM"))
        vd_pool = ctx.enter_context(tc.tile_pool(name="vd", bufs=2, space="PSUM"))
        sm_pool = ctx.enter_context(tc.tile_pool(name="smps", bufs=2, space="PSUM"))
        dram_pool = ctx.enter_context(tc.tile_pool(name="scratch", bufs=2, space="DRAM"))

        # --- small constants (cheap DMAs first) ---
        tgt_sb = consts.tile([P, DC * NB], F32)
        for dc in range(DC):
            nc.sync.dma_start(tgt_sb[:, dc * NB:(dc + 1) * NB], tgtT[dc * P:(dc + 1) * P, :])
        tgt_r = consts.tile([P, DC * NB], F32R)
        nc.vector.tensor_copy(tgt_r[:], tgt_sb[:])
        v_sb = consts.tile([P, ET], F32)
        nc.sync.dma_start(v_sb[:], vT[:, :])
        v_r = consts.tile([P, ET], F32R)
        nc.vector.tensor_copy(v_r[:], v_sb[:])
        keep_sb = consts.tile([P, NB * SQ], F32)
        for b in range(NB):
            nc.sync.dma_start(keep_sb[:, b * SQ:(b + 1) * SQ], keep[b])
        idx_sb = consts.tile([P, NB * nslot], U32)
        for b in range(NB):
            nc.sync.dma_start(
                idx_sb[:, b * nslot:(b + 1) * nslot],
                idxs[b].rearrange("slot p -> p slot"),
            )
        ones_sb = consts.tile([P, P], F32)
        nc.vector.memset(ones_sb[:], 1.0)
        zero_sb = consts.tile([P, (S + P) // P], F32)
        nc.vector.memset(zero_sb[:], 0.0)

        # --- weights: Wq first (the q matmuls below are first in PE order),
        # then Wk. The two f32 landing buffers share one pool slot (their
        # lifetimes are sequential) to stay inside SBUF.
        wq_r = consts.tile([P, DC * D], F32R)
        wq_sb = consts.tile([P, DC * D], F32, tag="wstage", name="wq_sb")
        for dc in range(DC):
            nc.sync.dma_start(wq_sb[:, dc * D:(dc + 1) * D], wqT[dc * P:(dc + 1) * P, :])
            nc.vector.tensor_copy(wq_r[:, dc * D:(dc + 1) * D], wq_sb[:, dc * D:(dc + 1) * D])
        wk_r = consts.tile([P, DC * D], F32R)
        wk_sb = consts.tile([P, DC * D], F32, tag="wstage", name="wk_sb")
        for dc in range(DC):
            nc.sync.dma_start(wk_sb[:, dc * D:(dc + 1) * D], wkT[dc * P:(dc + 1) * P, :])
            nc.vector.tensor_copy(wk_r[:, dc * D:(dc + 1) * D], wk_sb[:, dc * D:(dc + 1) * D])

        q_sb = consts.tile([P, NB * ET], F32)

        # q[b, e] = sum_d target[b, d] * Wq[e, d]: fp32r with target as the
        # M=2 stationary and WqT as the N=512 moving operand. The [2, 1024]
        # result is transposed into per-partition bias layout [128, 16]
        # (b-major columns) through a DRAM bounce.
        q_row = consts.tile([NB, D], F32)
        for j in range(D // SW):
            q_ps2 = sm_pool.tile([NB, SW], F32, tag="small", name="q_ps2")
            for dc in range(DC):
                nc.tensor.matmul(
                    q_ps2[:],
                    tgt_r[:, dc * NB:(dc + 1) * NB],
                    wq_r[:, dc * D + j * SW: dc * D + (j + 1) * SW],
                    start=(dc == 0),
                    stop=(dc == DC - 1),
                )
            nc.vector.tensor_copy(q_row[:, j * SW:(j + 1) * SW], q_ps2[:])
        qscr = dram_pool.tile([NB, D], F32, tag="qscr", name="qscr")
        nc.sync.dma_start(qscr[:], q_row[:])
        for b in range(NB):
            nc.sync.dma_start(
                q_sb[:, b * ET:(b + 1) * ET],
                qscr[b].rearrange("(et p) -> p et", p=P),
            )

        def emit_vd(vd_ps, tts, c, w):
            nc.tensor.matmul(
                vd_ps[:, :w],
                v_r[:, c:c + 1],
                tts[c][:, :w],
                start=(c == 0),
                stop=(c == ET - 1),
            )

        scrfs = []
        for b in range(NB):
            # exp strips land contiguously in compact scratch, each strip
            # scattered to its full-S positions right away (pads go to the
            # trash cell at S)
            scrf = dram_pool.tile([1, S + P], F32, tag="scrf", name="scrf")
            nc.sync.dma_start(scrf.rearrange("o (p f) -> (o p) f", p=P), zero_sb[:])
            scrfs.append(scrf)
            scratch_cb = dram_pool.tile([1, s_pad], F32, tag="scrc", name="scrc")
            off = 0
            for sp, w in enumerate(widths):
                mem_sb = mem_pool.tile([P, DC * SW], F32)
                mem_r = mem_pool.tile([P, DC * SW], F32R, tag="mem_r", name="mem_r")
                for dc in range(DC):
                    nc.sync.dma_start(
                        mem_sb[:, dc * SW:dc * SW + w],
                        memC[b, dc * P:(dc + 1) * P, off:off + w],
                    )
                    nc.vector.tensor_copy(
                        mem_r[:, dc * SW:dc * SW + w], mem_sb[:, dc * SW:dc * SW + w]
                    )
                vd_ps = vd_pool.tile([1, SW], F32, tag="vd", name="vd_ps")
                tts = []
                for et in range(ET):
                    k_ps = kps_pool.tile([P, SW], F32, tag="k", name="k_ps")
                    for dc in range(DC):
                        nc.tensor.matmul(
                            k_ps[:, :w],
                            wk_r[:, dc * D + et * P: dc * D + (et + 1) * P],
                            mem_r[:, dc * SW:dc * SW + w],
                            start=(dc == 0),
                            stop=(dc == DC - 1),
                        )
                    tt = tt_pool.tile([P, SW], F32R, tag="tt", name="tt")
                    nc.scalar.activation(
                        tt[:, :w], k_ps[:, :w], AF.Tanh,
                        bias=q_sb[:, b * ET + et: b * ET + et + 1],
                    )
                    tts.append(tt)
                    # keep the PE stream 2 e-tiles ahead of the v-dot so it
                    # never stalls waiting on the ACT tanh
                    if stage >= 2 and et >= 2:
                        emit_vd(vd_ps, tts, et - 2, w)
                if stage < 2:
                    if sp == len(widths) - 1:
                        dbg = fin_pool.tile([P, SQ], F32, tag="outt", name="dbg")
                        nc.vector.tensor_copy(dbg[:], tts[7][:, :SQ])
                        nc.sync.dma_start(out[b], dbg[:])
                    off += w
                    continue
                emit_vd(vd_ps, tts, ET - 2, w)
                emit_vd(vd_ps, tts, ET - 1, w)

                strip_sb = strip_pool.tile([1, SW], F32, tag="strip", name="strip_sb")
                nc.scalar.activation(strip_sb[:, :w], vd_ps[:, :w], AF.Exp)
                nc.sync.dma_start(scratch_cb[:, off:off + w], strip_sb[:, :w])
                # scatter this strip's exp values to their full-S positions.
                # HW consumes one offset per in_-contiguous descriptor run,
                # so arbitrary positions need [128, 1] single-element rows.
                f = w // P
                sc_sb = strip_pool.tile([P, SW // P], F32, tag="scsb", name="sc_sb", bufs=8)
                nc.sync.dma_start(
                    sc_sb[:, :f],
                    scratch_cb[:, off:off + w].rearrange("o (p f) -> (o p) f", f=f),
                )
                for jj in range(f):
                    col = b * nslot + (off // P) + jj
                    nc.gpsimd.indirect_dma_start(
                        out=scrf.rearrange("o (s w2) -> (o s) w2", w2=1),
                        out_offset=bass.IndirectOffsetOnAxis(
                            ap=idx_sb[:, col:col + 1], axis=0
                        ),
                        in_=sc_sb[:, jj:jj + 1],
                        in_offset=None,
                    )
                off += w

        # finales AFTER both batches' compute: the ones-matmuls are in PE
        # program order, so batch 0's finale must not sit between the two
        # batches' k-matmul streams (PE would stall on the scatter chain)
        for b in range(NB):
            if stage < 2:
                continue
            # --- masked softmax finale for batch b ---
            esq = fin_pool.tile([P, SQ], F32, tag="esq", name="esq")
            nc.sync.dma_start(
                esq[:], scrfs[b][:, :S].rearrange("o (p f) -> (o p) f", p=P)
            )
            if stage < 25:
                outt = fin_pool.tile([P, SQ], F32, tag="outt", name="outt")
                nc.vector.tensor_copy(outt[:], esq[:])
                nc.sync.dma_start(out[b], outt[:])
                continue
            em = fin_pool.tile([P, SQ], F32, tag="em", name="em")
            part = fin_pool.tile([P, 1], F32, tag="part", name="part")
            nc.vector.tensor_mul(em[:], esq[:], keep_sb[:, b * SQ:(b + 1) * SQ])
            nc.vector.reduce_sum(part[:], em[:], axis=mybir.AxisListType.X)
            if stage < 26:
                outt = fin_pool.tile([P, SQ], F32, tag="outt", name="outt")
                nc.vector.tensor_copy(outt[:], em[:])
                nc.sync.dma_start(out[b], outt[:])
                continue
            tot_ps = sm_pool.tile([P, 1], F32, tag="small", name="tot_ps")
            nc.tensor.matmul(tot_ps[:], ones_sb[:], part[:], start=True, stop=True)
            recip = fin_pool.tile([P, 1], F32, tag="recip", name="recip")
            nc.vector.reciprocal(recip[:], tot_ps[:])
            outt = fin_pool.tile([P, SQ], F32, tag="outt", name="outt")
            nc.vector.tensor_scalar_mul(outt[:], em[:], recip[:, 0:1])
            nc.sync.dma_start(out[b], outt[:])

    nc.compile()
    return nc


def get_program(stage=None, widths=None):
    if stage is None:
        stage = int(os.environ.get("KERNEL_STAGE", "27"))
    assert widths is not None
    key = (stage, widths)
    if key not in _CACHE:
        _CACHE[key] = _build_program(stage, widths)
    return _CACHE[key]


def prepare_in_maps(memory, target, memory_mask, Wq, Wk, v):
    memory = np.asarray(memory, dtype=np.float32)
    target = np.asarray(target, dtype=np.float32)
    Wq = np.asarray(Wq, dtype=np.float32)
    Wk = np.asarray(Wk, dtype=np.float32)
    v = np.asarray(v, dtype=np.float32)
    mask = np.asarray(memory_mask)

    # host-side sharding / layout prep (no arithmetic)
    keep_bool = ~mask                                                # [B, S]
    widths = strip_widths(int(keep_bool.sum(1).max()))
    s_pad = sum(widths)

    memT = memory.transpose(0, 2, 1)                                 # [B, D, S] view
    kept_pad = np.empty((B, s_pad), dtype=np.int64)
    scat_idx = np.empty((B, s_pad), dtype=np.int64)
    for b in range(B):
        k = np.flatnonzero(keep_bool[b])
        kept_pad[b, :len(k)] = k
        kept_pad[b, len(k):] = k[0]  # pad data: duplicate first kept column
        scat_idx[b, :len(k)] = k
        scat_idx[b, len(k):] = S     # pad scatter target: trash cell at S
    memC = np.empty((B, D, s_pad), dtype=np.float32)
    for b in range(B):
        memC[b] = memT[b][:, kept_pad[b]]

    # scatter offsets in per-strip slot order: strip of width w at compact
    # offset `off` bounces to SBUF [128, w/128] with element (p, jj) holding
    # compact position off + p*(w/128) + jj
    slot_list = []
    off = 0
    for w in widths:
        f = w // P
        block = scat_idx[:, off:off + w].reshape(B, P, f)
        for jj in range(f):
            slot_list.append(block[:, :, jj])
        off += w
    idxs = np.stack(slot_list, axis=1).astype(np.uint32)             # [B, nslot, P]

    wkT = np.ascontiguousarray(Wk.T)                                 # [D, D]
    wqT = np.ascontiguousarray(Wq.T)                                 # [D, D]
    tgtT = np.ascontiguousarray(target.T)                            # [D, B]
    vT = np.ascontiguousarray(v.reshape(ET, P).T)                    # [P, ET]
    keep = np.ascontiguousarray(
        keep_bool.astype(np.float32).reshape(B, P, SQ))              # [B, P, SQ]

    in_maps = [
        {
            "memC": np.ascontiguousarray(memC[c * NB:(c + 1) * NB]),
            "wkT": wkT,
            "wqT": wqT,
            "tgtT": np.ascontiguousarray(tgtT[:, c * NB:(c + 1) * NB]),
            "vT": vT,
            "keep": np.ascontiguousarray(keep[c * NB:(c + 1) * NB]),
            "idxs": np.ascontiguousarray(idxs[c * NB:(c + 1) * NB]),
        }
        for c in range(N_CORES)
    ]
    return in_maps, widths


def gather_output(results):
    out = np.empty((B, S), dtype=np.float32)
    for c in range(N_CORES):
        out[c * NB:(c + 1) * NB] = results[c]["out"].reshape(NB, S)
    return out


def kernel(memory, target, memory_mask, Wq, Wk, v):
    from concourse.bass_utils import run_bass_kernel_spmd

    in_maps, widths = prepare_in_maps(memory, target, memory_mask, Wq, Wk, v)
    nc = get_program(widths=widths)
    res = run_bass_kernel_spmd(nc, in_maps, list(range(N_CORES)))
    return gather_output(res.results)



# revision 5
# speedup vs baseline: 1.6427x; 1.6427x over previous
"""Additive (Bahdanau) attention scoring kernel for Trainium2, 8-core SPMD.

Reference computation (B=16, S=4096, D=1024, all fp32):
    q      = target @ Wq.T                    # [B, D]
    k      = memory @ Wk.T                    # [B, S, D]
    scores = tanh(q[:, None, :] + k) @ v      # [B, S]
    out    = softmax(scores - 1e9 * mask, axis=-1)

Sharding: batch across the 8 cores (2 batches per core), weights replicated.

Host-side prep is layout/quantize only (no arithmetic): memory is
transposed to [D, S] per batch, compacted to the unmasked positions
(masked positions contribute exactly 0 to the softmax since exp(-1e9)
underflows to 0 in fp32, so dropping them is algebraically exact),
cast to bf16, and pre-tiled so each DMA is one contiguous block.

Device layout ("s on partitions"): compact positions are processed in
s-tiles of 128. For each s-tile the PE computes k^T as
  k_ps[s=128, e=1024] += memtile[d=128, s=128].T @ WkT[d=128, e]
accumulated over the 8 d-chunks (16 matmuls of N=512, mem stationary,
Wk moving, all bf16). Everything else runs off the PE:
  - DVE adds q (materialized once per batch as a [128, 1024] tile via a
    K=1 ones-matmul against the on-device q = Wq-matmul result),
  - ACT applies tanh (fp32 PSUM -> bf16 SBUF),
  - DVE scalar_tensor_tensor multiplies by the broadcast v and its
    accum_out (per-partition free-axis sum) yields the 128 scores.
Finale per batch: add a 0/-1e9 pad-penalty tile (pads -> exp == 0),
ACT Exp with accum_out -> row sums, one 128x128 ones-matmul
reduces+broadcasts the total, DVE reciprocal + scale, DMA out the
compact [128, ST] probabilities. The host scatters them to full S
(pure indexing; masked positions are exactly 0).
"""

import math
import os
from contextlib import ExitStack

import ml_dtypes
import numpy as np

import concourse.tile as tile
from concourse import bacc, mybir
import concourse.bass as bass  # noqa: F401  (kept for parity with harness imports)

B, S, D = 16, 4096, 1024
N_CORES = 8
NB = B // N_CORES  # batches per core
P = 128
DC = D // P        # contraction chunks
EH = D // 512      # moving-operand halves (PSUM bank = 512 fp32)

F32 = mybir.dt.float32
BF16 = mybir.dt.bfloat16
AF = mybir.ActivationFunctionType
MUL = mybir.AluOpType.mult

BF16NP = ml_dtypes.bfloat16

_CACHE = {}


def _chunks(ST):
    """DMA chunks of up to 4 s-tiles (1 MiB of bf16 per full chunk)."""
    return [(i, min(4, ST - i)) for i in range(0, ST, 4)]


def _build_program(ST, stage):
    s_pad = ST * P
    chunks = _chunks(ST)

    nc = bacc.Bacc("TRN2", target_bir_lowering=False, debug=False)

    memC = nc.dram_tensor("memC", [NB, P, DC * s_pad], BF16, kind="ExternalInput").ap()
    wkT = nc.dram_tensor("wkT", [P, DC * D], BF16, kind="ExternalInput").ap()
    wqT = nc.dram_tensor("wqT", [P, DC * D], BF16, kind="ExternalInput").ap()
    tgtT = nc.dram_tensor("tgtT", [P, DC * NB], BF16, kind="ExternalInput").ap()
    vbc = nc.dram_tensor("vbc", [P, D], BF16, kind="ExternalInput").ap()
    pen = nc.dram_tensor("pen", [NB, P, ST], F32, kind="ExternalInput").ap()
    out = nc.dram_tensor("out", [NB, P, ST], F32, kind="ExternalOutput").ap()

    with tile.TileContext(nc) as tc, ExitStack() as ctx:
        consts = ctx.enter_context(tc.tile_pool(name="consts", bufs=1))
        mem_pool = ctx.enter_context(tc.tile_pool(name="mem", bufs=3))
        ti_pool = ctx.enter_context(tc.tile_pool(name="ti", bufs=3))
        tt_pool = ctx.enter_context(tc.tile_pool(name="tt", bufs=3))
        ttv_pool = ctx.enter_context(tc.tile_pool(name="ttv", bufs=2))
        fin_pool = ctx.enter_context(tc.tile_pool(name="fin", bufs=2))
        kps_pool = ctx.enter_context(tc.tile_pool(name="kps", bufs=3, space="PSUM"))
        sps_pool = ctx.enter_context(tc.tile_pool(name="sps", bufs=2, space="PSUM"))
        dram_pool = ctx.enter_context(tc.tile_pool(name="scratch", bufs=1, space="DRAM"))

        # --- constants / weights (issue order = DMA priority order) ---
        tgt_sb = consts.tile([P, DC * NB], BF16)
        nc.sync.dma_start(tgt_sb[:], tgtT[:, :])
        vbc_sb = consts.tile([P, D], BF16)
        nc.sync.dma_start(vbc_sb[:], vbc[:, :])
        pen_sb = consts.tile([P, NB * ST], F32)
        for b in range(NB):
            nc.sync.dma_start(pen_sb[:, b * ST:(b + 1) * ST], pen[b])
        wq_sb = consts.tile([P, DC * D], BF16)
        for dc in range(DC):
            nc.sync.dma_start(wq_sb[:, dc * D:(dc + 1) * D], wqT[:, dc * D:(dc + 1) * D])
        wk_sb = consts.tile([P, DC * D], BF16)
        for dc in range(DC):
            nc.sync.dma_start(wk_sb[:, dc * D:(dc + 1) * D], wkT[:, dc * D:(dc + 1) * D])

        ones1 = consts.tile([1, P], F32)
        nc.vector.memset(ones1[:], 1.0)
        ones128 = consts.tile([P, P], F32)
        nc.vector.memset(ones128[:], 1.0)

        q_sb = consts.tile([NB, D], F32)
        qt_sb = consts.tile([P, NB * D], F32)
        score_sb = consts.tile([P, NB * ST], F32)
        rs_sb = consts.tile([P, NB], F32)
        rc_sb = consts.tile([P, NB], F32)

        # --- q = target @ Wq.T on the PE, fp32 accumulate ---
        q_ps = kps_pool.tile([P, D], F32, tag="kps", name="q_ps")
        for eh in range(EH):
            for dc in range(DC):
                nc.tensor.matmul(
                    q_ps[0:NB, eh * 512:(eh + 1) * 512],
                    tgt_sb[:, dc * NB:(dc + 1) * NB],
                    wq_sb[:, dc * D + eh * 512: dc * D + (eh + 1) * 512],
                    start=(dc == 0),
                    stop=(dc == DC - 1),
                )
        nc.vector.tensor_copy(q_sb[:], q_ps[0:NB, :])

        # q_tile[b]: [128, 1024] with every partition row equal to q[b]
        # (rank-1 ones-matmul: K=1 stationary ones, q row moving). The q row
        # bounces through DRAM so each batch's row sits at base partition 0
        # (the moving operand must start at partition 0/32/64).
        qscr = dram_pool.tile([NB, D], F32, tag="qscr", name="qscr")
        nc.sync.dma_start(qscr[:], q_sb[:])
        for b in range(NB):
            qrow = consts.tile([1, D], F32, tag=f"qrow{b}", name=f"qrow{b}")
            nc.sync.dma_start(qrow[:], qscr[b:b + 1, :])
            qt_ps = kps_pool.tile([P, D], F32, tag="kps", name="qt_ps")
            for eh in range(EH):
                nc.tensor.matmul(
                    qt_ps[:, eh * 512:(eh + 1) * 512],
                    ones1[:, 0:P],
                    qrow[:, eh * 512:(eh + 1) * 512],
                    start=True,
                    stop=True,
                )
            nc.vector.tensor_copy(qt_sb[:, b * D:(b + 1) * D], qt_ps[:])

        # --- main loop: per batch, per DMA chunk, per 128-position s-tile ---
        for b in range(NB):
            for (coff, cnt) in chunks:
                w = cnt * P
                mem_sb = mem_pool.tile([P, DC * 4 * P], BF16, tag="mem", name="mem_sb")
                nc.sync.dma_start(
                    mem_sb[:, :DC * w],
                    memC[b, :, DC * coff * P: DC * coff * P + DC * w],
                )
                for t in range(cnt):
                    j = coff + t
                    k_ps = kps_pool.tile([P, D], F32, tag="kps", name="k_ps")
                    for dc in range(DC):
                        for eh in range(EH):
                            nc.tensor.matmul(
                                k_ps[:, eh * 512:(eh + 1) * 512],
                                mem_sb[:, dc * w + t * P: dc * w + (t + 1) * P],
                                wk_sb[:, dc * D + eh * 512: dc * D + (eh + 1) * 512],
                                start=(dc == 0),
                                stop=(dc == DC - 1),
                            )
                    ti = ti_pool.tile([P, D], F32, tag="ti", name="ti")
                    nc.vector.tensor_add(ti[:], k_ps[:], qt_sb[:, b * D:(b + 1) * D])
                    tt = tt_pool.tile([P, D], BF16, tag="tt", name="tt")
                    nc.scalar.activation(tt[:], ti[:], AF.Tanh)
                    ttv = ttv_pool.tile([P, D], BF16, tag="ttv", name="ttv")
                    nc.vector.scalar_tensor_tensor(
                        ttv[:], tt[:], 1.0, vbc_sb[:],
                        op0=MUL, op1=MUL,
                        accum_out=score_sb[:, b * ST + j: b * ST + j + 1],
                    )

        # --- finales (after both batches so the PE stream stays dense) ---
        for b in range(NB):
            sl = slice(b * ST, (b + 1) * ST)
            sm = fin_pool.tile([P, ST], F32, tag="sm", name="sm")
            nc.vector.tensor_add(sm[:], score_sb[:, sl], pen_sb[:, sl])
            if stage < 2:
                nc.sync.dma_start(out[b], sm[:])
                continue
            ex = fin_pool.tile([P, ST], F32, tag="ex", name="ex")
            nc.scalar.activation(ex[:], sm[:], AF.Exp, accum_out=rs_sb[:, b:b + 1])
            tot_ps = sps_pool.tile([P, 1], F32, tag="tot", name="tot_ps")
            nc.tensor.matmul(tot_ps[:], ones128[:], rs_sb[:, b:b + 1], start=True, stop=True)
            nc.vector.reciprocal(rc_sb[:, b:b + 1], tot_ps[:])
            ot = fin_pool.tile([P, ST], F32, tag="ot", name="ot")
            nc.vector.tensor_scalar_mul(ot[:], ex[:], rc_sb[:, b:b + 1])
            nc.sync.dma_start(out[b], ot[:])

    nc.compile()
    return nc


def get_program(ST=None, stage=None):
    if stage is None:
        stage = int(os.environ.get("KERNEL_STAGE", "2"))
    assert ST is not None
    key = (ST, stage)
    if key not in _CACHE:
        _CACHE[key] = _build_program(ST, stage)
    return _CACHE[key]


def prepare_in_maps(memory, target, memory_mask, Wq, Wk, v):
    memory = np.asarray(memory, dtype=np.float32)
    target = np.asarray(target, dtype=np.float32)
    Wq = np.asarray(Wq, dtype=np.float32)
    Wk = np.asarray(Wk, dtype=np.float32)
    v = np.asarray(v, dtype=np.float32)
    mask = np.asarray(memory_mask)

    keep = ~mask                                   # [B, S]
    counts = keep.sum(1).astype(np.int64)
    max_kept = int(counts.max())
    ST = math.ceil(max_kept / P)
    s_pad = ST * P
    chunks = _chunks(ST)

    kept_idx = []
    memC = np.empty((B, P, DC * s_pad), dtype=BF16NP)
    for b in range(B):
        idx = np.flatnonzero(keep[b])
        kept_idx.append(idx)
        pad = np.empty(s_pad, dtype=np.int64)
        pad[:len(idx)] = idx
        pad[len(idx):] = idx[0]
        # [D, s_pad] -> [P, DC, s_pad] (partition = d % 128) -> chunk-major
        A = memory[b][pad].T.astype(BF16NP).reshape(DC, P, s_pad).transpose(1, 0, 2)
        blocks = [
            np.ascontiguousarray(A[:, :, off * P:(off + cnt) * P]).reshape(P, DC * cnt * P)
            for (off, cnt) in chunks
        ]
        memC[b] = np.concatenate(blocks, axis=1)

    def chunked_T(W):  # [D, D] -> [P, DC*D] with partition = d % 128
        return np.ascontiguousarray(
            W.T.astype(BF16NP).reshape(DC, P, D).transpose(1, 0, 2).reshape(P, DC * D)
        )

    wkT = chunked_T(Wk)
    wqT = chunked_T(Wq)
    tgtT_full = target.T.astype(BF16NP).reshape(DC, P, B).transpose(1, 0, 2)  # [P, DC, B]
    vbc_arr = np.ascontiguousarray(np.broadcast_to(v.astype(BF16NP), (P, D)))

    # pad penalty: position j*128 + p is real iff < counts[b]
    pos = (np.arange(ST)[None, :] * P + np.arange(P)[:, None])  # [P, ST]
    pen = np.where(pos[None, :, :] < counts[:, None, None], 0.0, -1e9).astype(np.float32)

    in_maps = [
        {
            "memC": np.ascontiguousarray(memC[c * NB:(c + 1) * NB]),
            "wkT": wkT,
            "wqT": wqT,
            "tgtT": np.ascontiguousarray(
                tgtT_full[:, :, c * NB:(c + 1) * NB].reshape(P, DC * NB)
            ),
            "vbc": vbc_arr,
            "pen": np.ascontiguousarray(pen[c * NB:(c + 1) * NB]),
        }
        for c in range(N_CORES)
    ]
    meta = {"ST": ST, "counts": counts, "kept_idx": kept_idx}
    return in_maps, meta


def gather_output(results, meta):
    ST = meta["ST"]
    out = np.zeros((B, S), dtype=np.float32)
    for c in range(N_CORES):
        arr = np.asarray(results[c]["out"], dtype=np.float32)  # [NB, P, ST]
        for i in range(NB):
            b = c * NB + i
            compact = arr[i].T.reshape(ST * P)  # position j*128+p at [p, j]
            idx = meta["kept_idx"][b]
            out[b, idx] = compact[:len(idx)]
    return out


def kernel(memory, target, memory_mask, Wq, Wk, v):
    from concourse.bass_utils import run_bass_kernel_spmd

    in_maps, meta = prepare_in_maps(memory, target, memory_mask, Wq, Wk, v)
    nc = get_program(ST=meta["ST"])
    res = run_bass_kernel_spmd(nc, in_maps, list(range(N_CORES)))
    return gather_output(res.results, meta)


# revision 19
# speedup vs baseline: 1.7517x; 1.0663x over previous
"""Additive (Bahdanau) attention scoring kernel for Trainium2, 8-core SPMD.

Reference computation (B=16, S=4096, D=1024, all fp32):
    q      = target @ Wq.T                    # [B, D]
    k      = memory @ Wk.T                    # [B, S, D]
    scores = tanh(q[:, None, :] + k) @ v      # [B, S]
    out    = softmax(scores - 1e9 * mask, axis=-1)

Sharding: batch across the 8 cores (2 batches per core), weights replicated.

Host-side prep is layout/quantize only (no arithmetic): memory is
transposed to [D, S] per batch, compacted to the unmasked positions
(masked positions contribute exactly 0 to the softmax since exp(-1e9)
underflows to 0 in fp32, so dropping them is algebraically exact),
cast to bf16, and pre-tiled so each DMA is one contiguous block.

Device layout ("s on partitions"): compact positions are processed in
s-tiles of 128. For each s-tile the PE computes k^T as
  k_ps[s=128, e=1024] += memtile[d=128, s=128].T @ WkT[d=128, e]
accumulated over the 8 d-chunks (16 matmuls of N=512, mem stationary,
Wk moving, all bf16). Everything else runs off the PE:
  - DVE adds q (materialized once per batch as a [128, 1024] tile via a
    K=1 ones-matmul against the on-device q = Wq-matmul result),
  - ACT applies tanh (fp32 PSUM -> bf16 SBUF),
  - DVE scalar_tensor_tensor multiplies by the broadcast v and its
    accum_out (per-partition free-axis sum) yields the 128 scores.
Finale per batch: add a 0/-1e9 pad-penalty tile (pads -> exp == 0),
ACT Exp with accum_out -> row sums, one 128x128 ones-matmul
reduces+broadcasts the total, DVE reciprocal + scale, DMA out the
compact [128, ST] probabilities. The host scatters them to full S
(pure indexing; masked positions are exactly 0).
"""

import math
import os
from contextlib import ExitStack

import ml_dtypes
import numpy as np

import concourse.tile as tile
from concourse import bacc, mybir
import concourse.bass as bass  # noqa: F401  (kept for parity with harness imports)

B, S, D = 16, 4096, 1024
N_CORES = 8
NB = B // N_CORES  # batches per core
P = 128
DC = D // P        # contraction chunks
EH = D // 512      # moving-operand halves (PSUM bank = 512 fp32)

F32 = mybir.dt.float32
BF16 = mybir.dt.bfloat16
AF = mybir.ActivationFunctionType
MUL = mybir.AluOpType.mult

BF16NP = ml_dtypes.bfloat16

_CACHE = {}


def _chunks(ST):
    """DMA chunks of up to 4 s-tiles (1 MiB of bf16 per full chunk)."""
    return [(i, min(4, ST - i)) for i in range(0, ST, 4)]


def _build_program(ST, stage):
    s_pad = ST * P
    chunks = _chunks(ST)

    nc = bacc.Bacc("TRN2", target_bir_lowering=False, debug=False)

    memC = nc.dram_tensor("memC", [NB, P, DC * s_pad], BF16, kind="ExternalInput").ap()
    wkT = nc.dram_tensor("wkT", [P, DC * D], BF16, kind="ExternalInput").ap()
    wqT = nc.dram_tensor("wqT", [P, DC * D], BF16, kind="ExternalInput").ap()
    tgtT = nc.dram_tensor("tgtT", [P, DC * NB], BF16, kind="ExternalInput").ap()
    vbc = nc.dram_tensor("vbc", [P, D], BF16, kind="ExternalInput").ap()
    pen = nc.dram_tensor("pen", [NB, P, ST], F32, kind="ExternalInput").ap()
    sel = nc.dram_tensor("sel", [NB, NB * P], F32, kind="ExternalInput").ap()
    out = nc.dram_tensor("out", [NB, P, ST], F32, kind="ExternalOutput").ap()

    with tile.TileContext(nc) as tc, ExitStack() as ctx:
        consts = ctx.enter_context(tc.tile_pool(name="consts", bufs=1))
        mem_pool = ctx.enter_context(tc.tile_pool(name="mem", bufs=3))
        ti_pool = ctx.enter_context(tc.tile_pool(name="ti", bufs=3))
        tt_pool = ctx.enter_context(tc.tile_pool(name="tt", bufs=3))
        ttv_pool = ctx.enter_context(tc.tile_pool(name="ttv", bufs=2))
        fin_pool = ctx.enter_context(tc.tile_pool(name="fin", bufs=2))
        kps_pool = ctx.enter_context(tc.tile_pool(name="kps", bufs=3, space="PSUM"))
        qps_pool = ctx.enter_context(tc.tile_pool(name="qps", bufs=1, space="PSUM"))

        # --- constants / weights (issue order = DMA priority order: the
        # first memC chunk + Wk gate the PE's first k-matmuls; Wq is only
        # needed ~14us in; vbc/pen even later) ---
        tgt_sb = consts.tile([P, DC * NB], BF16)
        nc.sync.dma_start(tgt_sb[:], tgtT[:, :])
        sel_sb = consts.tile([NB, NB * P], F32)
        nc.sync.dma_start(sel_sb[:], sel[:, :])
        w0 = chunks[0][1] * P
        mem0_sb = mem_pool.tile([P, DC * 4 * P], BF16, tag="mem", name="mem_sb")
        nc.sync.dma_start(mem0_sb[:, :DC * w0], memC[0, :, 0:DC * w0])
        wk_sb = consts.tile([P, DC * D], BF16)
        for dc in range(DC):
            nc.sync.dma_start(wk_sb[:, dc * D:(dc + 1) * D], wkT[:, dc * D:(dc + 1) * D])
        wq_sb = consts.tile([P, DC * D], BF16)
        for dc in range(DC):
            nc.sync.dma_start(wq_sb[:, dc * D:(dc + 1) * D], wqT[:, dc * D:(dc + 1) * D])
        vbc_sb = consts.tile([P, D], BF16)
        nc.sync.dma_start(vbc_sb[:], vbc[:, :])
        pen_sb = consts.tile([P, NB * ST], F32)
        for b in range(NB):
            nc.sync.dma_start(pen_sb[:, b * ST:(b + 1) * ST], pen[b])

        ones128 = consts.tile([P, P], F32)
        nc.vector.memset(ones128[:], 1.0)

        q_sb = consts.tile([NB, D], F32)
        qt_sb = consts.tile([P, NB * D], F32)
        score_sb = consts.tile([P, NB * ST], F32)
        rs_sb = consts.tile([P, NB], F32)
        rc_sb = consts.tile([P, NB], F32)

        def emit_qsetup():
            # q = target @ Wq.T on the PE, fp32 accumulate -> [NB, D]
            q_ps = qps_pool.tile([P, D], F32, tag="qps", name="q_ps")
            for eh in range(EH):
                for dc in range(DC):
                    nc.tensor.matmul(
                        q_ps[0:NB, eh * 512:(eh + 1) * 512],
                        tgt_sb[:, dc * NB:(dc + 1) * NB],
                        wq_sb[:, dc * D + eh * 512: dc * D + (eh + 1) * 512],
                        start=(dc == 0),
                        stop=(dc == DC - 1),
                    )
            nc.vector.tensor_copy(q_sb[:], q_ps[0:NB, :])
            # q_tile[b]: [128, 1024] with every row equal to q[b], via a K=2
            # selector matmul (row b of q_sb picked by the 0/1 selector, so
            # the moving operand stays at base partition 0).
            for b in range(NB):
                qt_ps = qps_pool.tile([P, D], F32, tag="qps", name="qt_ps")
                for eh in range(EH):
                    nc.tensor.matmul(
                        qt_ps[:, eh * 512:(eh + 1) * 512],
                        sel_sb[:, b * P:(b + 1) * P],
                        q_sb[0:NB, eh * 512:(eh + 1) * 512],
                        start=True,
                        stop=True,
                    )
                nc.vector.tensor_copy(qt_sb[:, b * D:(b + 1) * D], qt_ps[:])

        def emit_tile_mm(mem_sb, w, t):
            k_ps = kps_pool.tile([P, D], F32, tag="kps", name="k_ps")
            for dc in range(DC):
                for eh in range(EH):
                    nc.tensor.matmul(
                        k_ps[:, eh * 512:(eh + 1) * 512],
                        mem_sb[:, dc * w + t * P: dc * w + (t + 1) * P],
                        wk_sb[:, dc * D + eh * 512: dc * D + (eh + 1) * 512],
                        start=(dc == 0),
                        stop=(dc == DC - 1),
                    )
            return k_ps

        def emit_tile_post(b, k_ps, j):
            ti = ti_pool.tile([P, D], F32, tag="ti", name="ti")
            nc.vector.tensor_add(ti[:], k_ps[:], qt_sb[:, b * D:(b + 1) * D])
            tt = tt_pool.tile([P, D], BF16, tag="tt", name="tt")
            nc.scalar.activation(tt[:], ti[:], AF.Tanh)
            ttv = ttv_pool.tile([P, D], BF16, tag="ttv", name="ttv")
            nc.vector.scalar_tensor_tensor(
                ttv[:], tt[:], 1.0, vbc_sb[:],
                op0=MUL, op1=MUL,
                accum_out=score_sb[:, b * ST + j: b * ST + j + 1],
            )

        # --- main loop. The first 3 tiles' k-matmuls (= kps pool depth) are
        # emitted before the q-setup so the PE can start as soon as Wk +
        # memC chunk 0 land; their DVE/ACT post-processing (which needs
        # q_tile) follows the q-setup. ---
        exs = []
        for b in range(NB):
            for ci, (coff, cnt) in enumerate(chunks):
                w = cnt * P
                if b == 0 and ci == 0:
                    mem_sb = mem0_sb
                    lead = min(3, cnt)
                    leads = [emit_tile_mm(mem_sb, w, t) for t in range(lead)]
                    emit_qsetup()
                    for t in range(lead):
                        emit_tile_post(b, leads[t], coff + t)
                    for t in range(lead, cnt):
                        emit_tile_post(b, emit_tile_mm(mem_sb, w, t), coff + t)
                    continue
                mem_sb = mem_pool.tile([P, DC * 4 * P], BF16, tag="mem", name="mem_sb")
                nc.sync.dma_start(
                    mem_sb[:, :DC * w],
                    memC[b, :, DC * coff * P: DC * coff * P + DC * w],
                )
                for t in range(cnt):
                    emit_tile_post(b, emit_tile_mm(mem_sb, w, t), coff + t)
            # per-batch finale front half (DVE/ACT only, so it interleaves
            # into the other batch's compute; the PE part comes at the end)
            sl = slice(b * ST, (b + 1) * ST)
            sm = fin_pool.tile([P, ST], F32, tag=f"sm{b}", name=f"sm{b}")
            nc.vector.tensor_add(sm[:], score_sb[:, sl], pen_sb[:, sl])
            if stage >= 2:
                ex = fin_pool.tile([P, ST], F32, tag=f"ex{b}", name=f"ex{b}")
                nc.scalar.activation(ex[:], sm[:], AF.Exp, accum_out=rs_sb[:, b:b + 1])
                exs.append(ex)
            else:
                exs.append(sm)

        # --- finale back half (the only PE/DVE work after the last k-matmul)
        for b in range(NB):
            if stage < 2:
                nc.sync.dma_start(out[b], exs[b][:])
                continue
            tot_ps = kps_pool.tile([P, D], F32, tag="kps", name="tot_ps")
            nc.tensor.matmul(tot_ps[:, 0:1], ones128[:], rs_sb[:, b:b + 1], start=True, stop=True)
            nc.vector.reciprocal(rc_sb[:, b:b + 1], tot_ps[:, 0:1])
            ot = fin_pool.tile([P, ST], F32, tag=f"ot{b}", name=f"ot{b}")
            nc.vector.tensor_scalar_mul(ot[:], exs[b][:], rc_sb[:, b:b + 1])
            nc.sync.dma_start(out[b], ot[:])

    nc.compile()
    return nc


def get_program(ST=None, stage=None):
    if stage is None:
        stage = int(os.environ.get("KERNEL_STAGE", "2"))
    assert ST is not None
    key = (ST, stage)
    if key not in _CACHE:
        _CACHE[key] = _build_program(ST, stage)
    return _CACHE[key]


def prepare_in_maps(memory, target, memory_mask, Wq, Wk, v):
    memory = np.asarray(memory, dtype=np.float32)
    target = np.asarray(target, dtype=np.float32)
    Wq = np.asarray(Wq, dtype=np.float32)
    Wk = np.asarray(Wk, dtype=np.float32)
    v = np.asarray(v, dtype=np.float32)
    mask = np.asarray(memory_mask)

    keep = ~mask                                   # [B, S]
    counts = keep.sum(1).astype(np.int64)
    max_kept = int(counts.max())
    ST = math.ceil(max_kept / P)
    s_pad = ST * P
    chunks = _chunks(ST)

    kept_idx = []
    memC = np.empty((B, P, DC * s_pad), dtype=BF16NP)
    for b in range(B):
        idx = np.flatnonzero(keep[b])
        kept_idx.append(idx)
        pad = np.empty(s_pad, dtype=np.int64)
        pad[:len(idx)] = idx
        pad[len(idx):] = idx[0]
        # [D, s_pad] -> [P, DC, s_pad] (partition = d % 128) -> chunk-major
        A = memory[b][pad].T.astype(BF16NP).reshape(DC, P, s_pad).transpose(1, 0, 2)
        blocks = [
            np.ascontiguousarray(A[:, :, off * P:(off + cnt) * P]).reshape(P, DC * cnt * P)
            for (off, cnt) in chunks
        ]
        memC[b] = np.concatenate(blocks, axis=1)

    def chunked_T(W):  # [D, D] -> [P, DC*D] with partition = d % 128
        return np.ascontiguousarray(
            W.T.astype(BF16NP).reshape(DC, P, D).transpose(1, 0, 2).reshape(P, DC * D)
        )

    wkT = chunked_T(Wk)
    wqT = chunked_T(Wq)
    tgtT_full = target.T.astype(BF16NP).reshape(DC, P, B).transpose(1, 0, 2)  # [P, DC, B]
    vbc_arr = np.ascontiguousarray(np.broadcast_to(v.astype(BF16NP), (P, D)))

    # pad penalty: position j*128 + p is real iff < counts[b]
    pos = (np.arange(ST)[None, :] * P + np.arange(P)[:, None])  # [P, ST]
    pen = np.where(pos[None, :, :] < counts[:, None, None], 0.0, -1e9).astype(np.float32)

    # batch-row selector for the q_tile matmul: sel[i, b*P + s] = (i == b)
    sel = np.zeros((NB, NB * P), dtype=np.float32)
    for b in range(NB):
        sel[b, b * P:(b + 1) * P] = 1.0

    in_maps = [
        {
            "memC": np.ascontiguousarray(memC[c * NB:(c + 1) * NB]),
            "wkT": wkT,
            "wqT": wqT,
            "tgtT": np.ascontiguousarray(
                tgtT_full[:, :, c * NB:(c + 1) * NB].reshape(P, DC * NB)
            ),
            "vbc": vbc_arr,
            "pen": np.ascontiguousarray(pen[c * NB:(c + 1) * NB]),
            "sel": sel,
        }
        for c in range(N_CORES)
    ]
    meta = {"ST": ST, "counts": counts, "kept_idx": kept_idx}
    return in_maps, meta


def gather_output(results, meta):
    ST = meta["ST"]
    out = np.zeros((B, S), dtype=np.float32)
    for c in range(N_CORES):
        arr = np.asarray(results[c]["out"], dtype=np.float32)  # [NB, P, ST]
        for i in range(NB):
            b = c * NB + i
            compact = arr[i].T.reshape(ST * P)  # position j*128+p at [p, j]
            idx = meta["kept_idx"][b]
            out[b, idx] = compact[:len(idx)]
    return out


def kernel(memory, target, memory_mask, Wq, Wk, v):
    from concourse.bass_utils import run_bass_kernel_spmd

    in_maps, meta = prepare_in_maps(memory, target, memory_mask, Wq, Wk, v)
    nc = get_program(ST=meta["ST"])
    res = run_bass_kernel_spmd(nc, in_maps, list(range(N_CORES)))
    return gather_output(res.results, meta)
